# revision 8
# baseline (speedup 1.0000x reference)
"""v4.5: single-stream elementwise Stormer-Cowell(6) kernel.

Math: integrate theta'' = -omega0^2 sin(theta) + coupling*(ring laplacian)
in rescaled units tau = omega0*t, angles in TURNS (y = theta/2pi):
    y'' = cp*lap(y) - sin(2pi y)/(2pi),   cp = coupling/omega0^2.

Integrator: explicit Stormer-Cowell order 6 (1 force eval per step):
    y_{n+1} = 2 y_n - y_{n-1}
              + (h^2/240)(299 f_n - 176 f_{n-1} + 194 f_{n-2}
                          - 96 f_{n-3} + 19 f_{n-4})
bootstrapped by 4 classical RKN4 steps (which also fill the f history).
Velocities are never needed after bootstrap (only final angles output).

Layout: pure data parallel, 128 batch rows per core on partitions, the
full 512-site ring contiguous on the free dim in padded [128, 514]
tiles (col 0 = ring 511, cols 1..512 = ring, col 513 = ring 0). The
laplacian is two shifted-AP reads; the ring wrap is one strided 2-col
copy per step.

Engine placement (HW-measured: cross-engine semaphore waits cost
~0.5us each and dominated earlier split-engine variants): ALL
elementwise ops run on the DVE (custom AXPBY ~0.7ns/elem, same as
plain adds); only Sin runs on ACT -> exactly 2 cross-engine deps per
step (w->sin, sin->F), both hidden behind ~7 independent DVE ops.
Main loop is a 12-step For_i body (state rings have period 12).
"""

import math

import numpy as np

import concourse.bacc as bacc
import concourse.dve_ops as dve_ops
import concourse.mybir as mybir
import concourse.tile as tile
from concourse.bass_utils import run_bass_kernel_spmd
from concourse.dve_spec import C0, C1, C2, Spec, Src0, Src1, _has_src1, lower
from concourse.dve_uop import DveOpSpec

F32 = mybir.dt.float32
AF = mybir.ActivationFunctionType
OP = mybir.AluOpType

N_CORES = 8
B, N = 1024, 512
PB = B // N_CORES            # 128 batch rows per core (partition dim)
PW = N + 2                   # padded tile width (halo cols 0 and 513)

NSTEPS = 92                  # trig-fitted coefficients; HW rel err ~1.2e-2
NBOOT = 4                    # RKN4 bootstrap steps (fill f history)
BODY = 12                    # steps per For_i iteration (lcm of ring sizes)
T_END = 2.0
TWO_PI = 2 * math.pi
MAGIC = 12582912.0           # 1.5 * 2**23: fp32 round-to-nearest trick
DEN = 240.0
CST = (299.0, -176.0, 194.0, -96.0, 19.0)   # classical order-5 set (unused)

# Trigonometrically-fitted 5-tap coefficients: exact for {1, t, t^2} and for
# cos/sin at two band frequencies l1, l2 (tau units) instead of t^3..t^5.
# Kills the dominant phase error of the oscillatory modes -> same accuracy
# as the classical set with ~25% fewer steps (sim + HW verified).
FIT_L1, FIT_L2 = 0.3, 1.38


def _fit_cst(nsteps, omega0):
    import numpy as _np
    h = omega0 * T_END / nsteps
    A, b = [[1.0] * 5], [1.0]
    for lam in (FIT_L1, FIT_L2):
        nu = lam * h
        A.append([_np.cos(j * nu) for j in range(5)])
        b.append(2 * (1 - _np.cos(nu)) / nu ** 2)
        A.append([_np.sin(j * nu) for j in range(5)])
        b.append(0.0)
    c = _np.linalg.solve(_np.array(A), _np.array(b))
    return tuple(float(v) for v in c)

SIN_ON_DVE = False      # timing-only ablation: wrong math, no ACT round trip


def _register_custom_op(name, body, reference):
    for op in dve_ops.OPS:
        if op.name == name:
            return op
    idx = dve_ops._CUSTOM_DVE_ROW_BASE + len(dve_ops.OPS)
    assert idx < 0x20
    spec = Spec(body=body, reference=reference)
    shas = {}
    for ver in ("v3", "v4"):
        try:
            uops = lower(spec, ver=ver)
            tmp = DveOpSpec(name=name, opcode=idx, uops=uops,
                            rd1_en=_has_src1(spec))
            shas[ver] = tmp.sha(ver)
        except Exception:
            pass
    op = dve_ops.DveOp(name, spec, subdim=False, uops_sha=shas)
    dve_ops.OPS.append(op)
    dve_ops._SUB_OPCODE_FOR_NAME[name] = idx
    dve_ops.CUSTOM_DVE_SPECS[name] = spec
    return op


def _f32(v):
    return np.float32(v)


_tw_z = Src0 * C0 + Src1 * C1
TURNS_WRAP = _register_custom_op(
    "TURNS_WRAP_ANT",
    _tw_z - ((_tw_z + C2) - C2),
    lambda in0, in1, s0, s1, imm2: (
        lambda z: z - ((z + _f32(imm2)) - _f32(imm2)))(
        (in0.astype(np.float32) * _f32(s0)
         + in1.astype(np.float32) * _f32(s1)).astype(np.float32)),
)
AXPBY = _register_custom_op(
    "AXPBY_ANT",
    Src0 * C0 + Src1 * C1,
    lambda in0, in1, s0, s1, imm2: in0.astype(np.float32) * _f32(s0)
    + in1.astype(np.float32) * _f32(s1),
)


def _build(nsteps: int, omega0: float, coupling: float, body: int = BODY):
    # A fully-unrolled single pass (body == all main steps) has no ring-period
    # constraint; the For_i path needs body % 12 == 0 so the state rings
    # return to their start position every iteration.
    assert (nsteps - NBOOT) % body == 0, (nsteps, body)
    assert body == nsteps - NBOOT or body % 12 == 0, (nsteps, body)
    h = omega0 * T_END / nsteps
    cp = coupling / (omega0 * omega0)
    CST = _fit_cst(nsteps, omega0)    # den=1: fitted c_j are absolute
    s_lap = cp * h * h                # F = s_lap*lap + s_sin*ns
    s_sin = h * h / TWO_PI

    nc = bacc.Bacc("TRN2", target_bir_lowering=False, debug=False,
                   num_devices=N_CORES)
    x_in = nc.dram_tensor("x", [PB, N], F32, kind="ExternalInput")
    out = nc.dram_tensor("out", [PB, N], F32, kind="ExternalOutput")

    with tile.TileContext(nc) as tc:
        with (
            tc.tile_pool(name="state", bufs=1) as st,
            tc.tile_pool(name="tmp", bufs=2) as tp,
        ):
            yring = [st.tile([PB, PW], F32, name=f"y_{j}", tag=f"y_{j}")
                     for j in range(4)]
            fring = [st.tile([PB, N], F32, name=f"F_{j}", tag=f"F_{j}")
                     for j in range(6)]
            ub = [st.tile([PB, N], F32, name=f"u_{j}", tag=f"u_{j}")
                  for j in range(2)]
            qaring = [st.tile([PB, N], F32, name=f"qa_{j}", tag=f"qa_{j}")
                      for j in range(2)]
            qbring = [st.tile([PB, N], F32, name=f"qb_{j}", tag=f"qb_{j}")
                      for j in range(2)]

            def T(tag, w=N):
                return tp.tile([PB, w], F32, name=tag, tag=tag)

            def interior(y):
                return y[:, 1:N + 1]

            def halo(y):
                # cols (0, 513) <- cols (512, 1), one strided DVE copy
                nc.vector.tensor_copy(y[:, 0:PW:PW - 1],
                                      y[:, N:0:-(N - 1)])

            def wrap(w_out, in0, in1, s0, s1):
                nc.vector._custom_dve(TURNS_WRAP, out=w_out[:], in0=in0[:],
                                      in1=in1[:], s0=s0, s1=s1, imm2=MAGIC)

            def axpby(o, a, b, s0, s1):
                nc.vector._custom_dve(AXPBY, out=o[:], in0=a[:], in1=b[:],
                                      s0=s0, s1=s1)

            def lap_pair(y, tag):
                t1 = T(f"t1{tag}")
                t2 = T(f"t2{tag}")
                nc.vector.tensor_add(t1[:], y[:, 0:N], y[:, 2:N + 2])
                axpby(t2, interior(y), t1, -2.0, 1.0)
                return t2

            def sin_of(w, tag):
                ns = T(f"ns{tag}")
                if SIN_ON_DVE:
                    nc.vector.tensor_copy(ns[:], w[:])
                else:
                    nc.scalar.activation(ns[:], w[:], AF.Sin, scale=-TWO_PI)
                return ns

            # ---------------- init ----------------
            y0 = yring[0]
            stage = T("stage")
            nc.gpsimd.dma_start(stage[:], x_in[:])
            nc.scalar.activation(interior(y0), stage[:], AF.Copy,
                                 bias=-0.5, scale=1.0)
            nc.vector.memset(ub[0][:], 0.0)
            halo(y0)

            ys, yprev = y0, None
            uc, uo = ub[0], ub[1]

            # ------------- RKN4 bootstrap (emits F_{k-1} at step k) -------
            for k in range(1, NBOOT + 1):
                w1 = T("w")
                wrap(w1, interior(ys), interior(ys), 1.0, 0.0)
                ns1 = sin_of(w1, "")
                m1 = lap_pair(ys, "")
                axpby(fring[k - 1], m1, ns1, s_lap, s_sin)
                a1s = T("a1s")
                axpby(a1s, m1, ns1, cp * h * h / 8, h * h / (8 * TWO_PI))
                hu = T("hu")
                axpby(hu, uc, interior(ys), h / 2, 1.0)
                p2 = T("pp", PW)
                nc.vector.tensor_add(interior(p2), hu[:], a1s[:])
                halo(p2)
                w2 = T("w")
                wrap(w2, a1s, hu, 1.0, 1.0)
                ns2 = sin_of(w2, "")
                m2 = lap_pair(p2, "")
                a2s = T("a2s")
                axpby(a2s, m2, ns2, cp * h * h / 2, h * h / (2 * TWO_PI))
                t_ = T("t_")
                axpby(t_, uc, interior(ys), h, 1.0)
                p3 = T("pp", PW)
                nc.vector.tensor_add(interior(p3), t_[:], a2s[:])
                halo(p3)
                w3 = T("w")
                wrap(w3, a2s, t_, 1.0, 1.0)
                ns3 = sin_of(w3, "")
                m3 = lap_pair(p3, "")
                ynew = yring[k % 4]
                g2 = T("g2")
                axpby(g2, a1s, a2s, 4.0 / 3.0, 2.0 / 3.0)
                nc.vector.tensor_add(interior(ynew), t_[:], g2[:])
                halo(ynew)
                if k < NBOOT:
                    k3s = T("k3s")
                    axpby(k3s, m3, ns3, cp * h / 6, h / (12 * math.pi))
                    s12 = T("s12")
                    axpby(s12, a1s, a2s, 4.0 / (3.0 * h), 4.0 / (3.0 * h))
                    v = T("v")
                    nc.vector.tensor_add(v[:], s12[:], uc[:])
                    nc.vector.tensor_add(uo[:], k3s[:], v[:])
                    uc, uo = uo, uc
                yprev = ys
                ys = ynew

            # ------------- Stormer-Cowell(6) main loop -------------------
            # Software-pipelined: step n also computes qa/qb for step n+1
            # (pure F-history combinations), so every same-engine RAW pair
            # in the DVE stream has >=1 unrelated op between producer and
            # consumer (hides the SBUF write->read turnaround). The ring
            # wrap is a 2-element AXPBY from base/F (no dependent copy).
            def sc_step(n, ys, yprev):
                fn = fring[n % 6]
                f1 = fring[(n - 1) % 6]
                ynew = yring[(n + 1) % 4]
                qa_p = qaring[n % 2]
                qb_p = qbring[n % 2]

                w = T("w")
                wrap(w, interior(ys), interior(ys), 1.0, 0.0)
                ns = sin_of(w, "")
                t1 = T("t1")
                nc.vector.tensor_add(t1[:], ys[:, 0:N], ys[:, 2:N + 2])
                q1 = T("q1")
                axpby(q1, interior(ys), interior(yprev), 2.0, -1.0)
                t2 = T("t2")
                axpby(t2, interior(ys), t1, -2.0, 1.0)
                qh = T("qh")
                nc.vector.tensor_add(qh[:], qa_p[:], qb_p[:])
                axpby(fn, t2, ns, s_lap, s_sin)
                base = T("qs")
                nc.vector.tensor_add(base[:], q1[:], qh[:])
                axpby(qaring[(n + 1) % 2], fn, f1, CST[1], CST[2])
                axpby(qbring[(n + 1) % 2], fring[(n - 2) % 6],
                      fring[(n - 3) % 6], CST[3], CST[4])
                nc.vector._custom_dve(AXPBY, out=interior(ynew),
                                      in0=base[:], in1=fn[:],
                                      s0=1.0, s1=CST[0])
                # wrap cols: ynew[0] = ring 511, ynew[513] = ring 0
                nc.vector._custom_dve(
                    AXPBY, out=ynew[:, 0:PW:PW - 1],
                    in0=base[:, N - 1::-(N - 1)],
                    in1=fn[:, N - 1::-(N - 1)], s0=1.0, s1=CST[0])
                return ynew, ys

            # prologue: qa/qb for the first main step (n = NBOOT)
            axpby(qaring[NBOOT % 2], fring[(NBOOT - 1) % 6],
                  fring[(NBOOT - 2) % 6], CST[1], CST[2])
            axpby(qbring[NBOOT % 2], fring[(NBOOT - 3) % 6],
                  fring[(NBOOT - 4) % 6], CST[3], CST[4])

            niter = (nsteps - NBOOT) // body
            if niter == 1:
                for j in range(body):
                    ys, yprev = sc_step(NBOOT + j, ys, yprev)
            elif niter > 0:
                with tc.For_i(0, niter) as _:
                    for j in range(body):
                        ys, yprev = sc_step(NBOOT + j, ys, yprev)

            # ---------------- output ----------------
            rad = T("rad")
            nc.scalar.activation(rad[:], interior(ys), AF.Copy,
                                 bias=0.0, scale=TWO_PI)
            nc.gpsimd.dma_start(out[:], rad[:])

    nc.compile()
    return nc


_CACHE: dict = {}


def _auto_body(nsteps):
    # Full unroll whenever it fits (the unrolled stream hits ~94% DVE
    # streaming efficiency; For_i iterations measurably inflate per-step
    # cost); else the largest multiple-of-12 divisor up to 792.
    m = nsteps - NBOOT
    if m <= 792:
        return m
    best = 12
    for k in range(1, m // 12 + 1):
        b = 12 * k
        if b > 792:
            break
        if m % b == 0:
            best = b
    return best


def _get(nsteps, om, cpl, body=None):
    if body is None:
        body = _auto_body(nsteps)
    key = (nsteps, om, cpl, body, SIN_ON_DVE)
    if key not in _CACHE:
        _CACHE[key] = _build(nsteps, om, cpl, body)
    return _CACHE[key]


def kernel(x, omega0, coupling, nsteps: int = None):
    x = np.ascontiguousarray(np.asarray(x, dtype=np.float32))
    om = float(np.asarray(omega0, dtype=np.float64))
    cpl = float(np.asarray(coupling, dtype=np.float64))
    if nsteps is None:
        nsteps = NSTEPS
    nc = _get(nsteps, om, cpl)
    in_maps = [{"x": x[i * PB:(i + 1) * PB]} for i in range(N_CORES)]
    res = run_bass_kernel_spmd(nc, in_maps, list(range(N_CORES)))
    return np.concatenate([r["out"] for r in res.results],
                          axis=0).astype(np.float32)

